# revision 4
# baseline (speedup 1.0000x reference)
"""
CoordinationHistogram Trainium2 kernel.

Problem (per system s of 8): given 1M neighbor displacement vectors and the
atom index of each pair, compute per-atom coordination numbers
  coords[a] = sum_e z_e * [first_atom[e] == a]   (20000 atoms)
with z_e a smooth switching function of |v_e|, then a Gaussian-KDE histogram
  hist[k] = sum_a exp(-2 * (coords[a] - k)^2),  k = 0..15.

Sharding: S axis across the 8 NeuronCores (data parallel, one system/core).

Device algorithm (per core):
  Scatter-add via a two-level one-hot matmul. Split atom id a = q*128 + l
  (q = a >> 7 in [0,157), l = a & 127). For each tile of 128 edges build
    L[e, l] = z_e * (l == l_e)   [128 x 128] bf16  (one tensor_scalar op)
    H[e, q] = (q == q_e)         [128 x 157] bf16  (one tensor_scalar op)
  and accumulate  coords[l, q] += L^T @ H  into PSUM with the tensor engine.
  The KDE then reads coords from PSUM: 16x (Square, Exp+row-accum) on the
  scalar engine, followed by a ones-vector matmul for the partition reduce.
  Atoms 20000..20095 are phantom PSUM cells; the KDE reduction APs simply
  exclude them (q == 156 is only reduced over l < 32).
"""

import numpy as np

import concourse.tile as tile
from concourse import bacc, mybir
from concourse.bass_utils import run_bass_kernel_spmd

P = 128
NQ = 157            # ceil(20000 / 128)
NATOMS = 20000
K = 16
E = 1_000_000
NCOL_FULL = 7813    # ceil(E / 128); E_pad = 7813*128 = 1000064
TBLK = 512          # columns (of 128 edges) per DMA block

R1 = 4.4
INV2 = float(1.0 / (1.1 * 1.1))   # 1/(R0-R1)^2
PAD_ATOM = 20064    # phantom cell (l=96, q=156)

F32 = mybir.dt.float32
BF16 = mybir.dt.bfloat16
I32 = mybir.dt.int32
OP = mybir.AluOpType
AF = mybir.ActivationFunctionType


def build_nc(ncol=NCOL_FULL):
    """Build the per-core Bass module. Inputs are padded to ncol*128 edges."""
    e_pad = ncol * P
    nc = bacc.Bacc("TRN2", target_bir_lowering=False, debug=False)
    nv = nc.dram_tensor("nv", [e_pad * 3], F32, kind="ExternalInput")
    fa = nc.dram_tensor("fa", [e_pad], I32, kind="ExternalInput")
    out = nc.dram_tensor("out", [1, K], F32, kind="ExternalOutput")

    # block layout: blocks of TBLK columns (+ remainder block)
    blocks = []
    c = 0
    while c < ncol:
        tb = min(TBLK, ncol - c)
        blocks.append((c, tb))
        c += tb

    with tile.TileContext(nc) as tc:
        with (
            tc.tile_pool(name="const", bufs=1) as cpool,
            tc.tile_pool(name="io", bufs=2) as iopool,
            tc.tile_pool(name="work", bufs=2) as wpool,
            tc.tile_pool(name="mask", bufs=8) as mpool,
            tc.tile_pool(name="psum", bufs=1, space="PSUM") as ppool,
        ):
            iota_l = cpool.tile([P, P], BF16)
            nc.gpsimd.iota(iota_l[:], pattern=[[1, P]], base=0,
                           channel_multiplier=0,
                           allow_small_or_imprecise_dtypes=True)
            iota_q = cpool.tile([P, NQ], BF16)
            nc.gpsimd.iota(iota_q[:], pattern=[[1, NQ]], base=0,
                           channel_multiplier=0,
                           allow_small_or_imprecise_dtypes=True)
            ones = cpool.tile([P, 1], F32)
            nc.vector.memset(ones[:], 1.0)
            bias_m1 = cpool.tile([P, 1], F32)
            nc.vector.memset(bias_m1[:], -1.0)
            bias_k = cpool.tile([P, K], F32)
            for k in range(K):
                nc.vector.memset(bias_k[:, k:k + 1], float(-k))

            coords = ppool.tile([P, NQ], F32, space="PSUM")

            col = 0
            for (c0, tb) in blocks:
                ofs_e = c0 * P
                nvb = iopool.tile([P, TBLK * 3], F32, tag="nvb")
                fab = iopool.tile([P, TBLK], I32, tag="fab")
                nc.sync.dma_start(
                    nvb[:, : tb * 3],
                    nv[ofs_e * 3: (ofs_e + P * tb) * 3].rearrange(
                        "(p m) -> p m", p=P),
                )
                nc.sync.dma_start(
                    fab[:, :tb],
                    fa[ofs_e: ofs_e + P * tb].rearrange("(p m) -> p m", p=P),
                )
                v3 = nvb[:, : tb * 3].rearrange("p (m c) -> p m c", c=3)
                x, y, w = v3[:, :, 0], v3[:, :, 1], v3[:, :, 2]

                d2 = wpool.tile([P, TBLK], F32, tag="d2")
                t1 = wpool.tile([P, TBLK], F32, tag="t1")
                nc.vector.tensor_tensor(out=d2[:, :tb], in0=x, in1=x, op=OP.mult)
                nc.vector.tensor_tensor(out=t1[:, :tb], in0=y, in1=y, op=OP.mult)
                nc.vector.tensor_tensor(out=d2[:, :tb], in0=d2[:, :tb],
                                        in1=t1[:, :tb], op=OP.add)
                nc.vector.tensor_tensor(out=t1[:, :tb], in0=w, in1=w, op=OP.mult)
                nc.vector.tensor_tensor(out=d2[:, :tb], in0=d2[:, :tb],
                                        in1=t1[:, :tb], op=OP.add)
                # s = sqrt(d2) / 1.1
                sv = wpool.tile([P, TBLK], F32, tag="sv")
                nc.scalar.activation(sv[:, :tb], d2[:, :tb], AF.Sqrt, scale=INV2)
                # yc = clamp(s - 4, 0, 1)
                yc = wpool.tile([P, TBLK], F32, tag="yc")
                nc.vector.tensor_scalar(yc[:, :tb], sv[:, :tb], 4.0, 0.0,
                                        op0=OP.subtract, op1=OP.max)
                nc.vector.tensor_scalar(yc[:, :tb], yc[:, :tb], 1.0, None,
                                        op0=OP.min)
                # z = (yc-1)^2 * (2*yc + 1)
                vv = wpool.tile([P, TBLK], F32, tag="vv")
                nc.scalar.activation(vv[:, :tb], yc[:, :tb], AF.Square,
                                     bias=bias_m1[:])
                w2 = wpool.tile([P, TBLK], F32, tag="w2")
                nc.vector.tensor_scalar(w2[:, :tb], yc[:, :tb], 2.0, 1.0,
                                        op0=OP.mult, op1=OP.add)
                zf = wpool.tile([P, TBLK], F32, tag="zf")
                nc.vector.tensor_tensor(out=zf[:, :tb], in0=vv[:, :tb],
                                        in1=w2[:, :tb], op=OP.mult)
                # q = a >> 7, l = a & 127 (as f32 scalars for is_equal)
                qi = wpool.tile([P, TBLK], I32, tag="qi")
                ri = wpool.tile([P, TBLK], I32, tag="ri")
                nc.vector.tensor_scalar(qi[:, :tb], fab[:, :tb], 7, None,
                                        op0=OP.logical_shift_right)
                nc.vector.tensor_scalar(ri[:, :tb], fab[:, :tb], 127, None,
                                        op0=OP.bitwise_and)
                qf = wpool.tile([P, TBLK], F32, tag="qf")
                rf = wpool.tile([P, TBLK], F32, tag="rf")
                nc.vector.tensor_copy(qf[:, :tb], qi[:, :tb])
                nc.vector.tensor_copy(rf[:, :tb], ri[:, :tb])

                for t in range(tb):
                    lt = mpool.tile([P, P], BF16, tag="lt")
                    ht = mpool.tile([P, NQ], BF16, tag="ht")
                    nc.vector.tensor_scalar(lt[:], iota_l[:],
                                            rf[:, t:t + 1], zf[:, t:t + 1],
                                            op0=OP.is_equal, op1=OP.mult)
                    nc.vector.tensor_scalar(ht[:], iota_q[:],
                                            qf[:, t:t + 1], None,
                                            op0=OP.is_equal)
                    nc.tensor.matmul(out=coords[:], lhsT=lt[:], rhs=ht[:],
                                     start=(col == 0), stop=(col == ncol - 1))
                    col += 1

            # ---- KDE ----
            acc1 = cpool.tile([P, K], F32)
            acc2 = cpool.tile([32, K], F32)
            sq = wpool.tile([P, NQ], F32, tag="sq")
            ek = wpool.tile([P, NQ], F32, tag="ek")
            for k in range(K):
                nc.scalar.activation(sq[:], coords[:], AF.Square,
                                     bias=bias_k[:, k:k + 1])
                nc.scalar.activation(ek[:, : NQ - 1], sq[:, : NQ - 1], AF.Exp,
                                     scale=-2.0, accum_out=acc1[:, k:k + 1])
                nc.scalar.activation(ek[0:32, NQ - 1: NQ], sq[0:32, NQ - 1: NQ],
                                     AF.Exp, scale=-2.0,
                                     accum_out=acc2[:, k:k + 1])
            hist_ps = ppool.tile([1, K], F32, space="PSUM")
            nc.tensor.matmul(out=hist_ps[:], lhsT=ones[:], rhs=acc1[:],
                             start=True, stop=False)
            nc.tensor.matmul(out=hist_ps[:], lhsT=ones[0:32, :], rhs=acc2[:],
                             start=False, stop=True)
            res = cpool.tile([1, K], F32)
            nc.vector.tensor_copy(res[:], hist_ps[:])
            nc.sync.dma_start(out[:], res[:])
    nc.compile()
    return nc


def _shard_inputs(neighbor_vectors, first_atom, ncol=NCOL_FULL):
    """Slice per system, pad to ncol*128 edges. Padding: z=0 vector (d=10)
    and a phantom atom id, so padded edges contribute exactly nothing."""
    e_pad = ncol * P
    s = neighbor_vectors.shape[0]
    in_maps = []
    for i in range(s):
        nvs = np.asarray(neighbor_vectors[i], dtype=np.float32).reshape(-1, 3)
        fas = np.asarray(first_atom[i], dtype=np.int32).reshape(-1)
        n = min(e_pad, nvs.shape[0])
        nv_pad = np.empty((e_pad, 3), dtype=np.float32)
        nv_pad[:n] = nvs[:n]
        nv_pad[n:] = np.array([10.0, 0.0, 0.0], dtype=np.float32)
        fa_pad = np.full((e_pad,), PAD_ATOM, dtype=np.int32)
        fa_pad[:n] = fas[:n]
        in_maps.append({"nv": nv_pad.reshape(-1), "fa": fa_pad})
    return in_maps


def run(neighbor_vectors, first_atom, ncol=NCOL_FULL, trace=False):
    nc = build_nc(ncol)
    in_maps = _shard_inputs(neighbor_vectors, first_atom, ncol)
    br = run_bass_kernel_spmd(nc, in_maps, core_ids=list(range(len(in_maps))),
                              trace=trace)
    out = np.stack([br.results[i]["out"][0] for i in range(len(in_maps))])
    return out.astype(np.float32), br


def kernel(neighbor_vectors, first_atom):
    out, _ = run(neighbor_vectors, first_atom)
    return out


# revision 13
# speedup vs baseline: 76.6324x; 76.6324x over previous
"""
CoordinationHistogram Trainium2 kernel.

Problem (per system s of 8): given 1M neighbor displacement vectors and the
atom index of each pair, compute per-atom coordination numbers
  coords[a] = sum_e z_e * [first_atom[e] == a]   (20000 atoms)
with z_e a smooth switching function of |v_e|, then a Gaussian-KDE histogram
  hist[k] = sum_a exp(-2 * (coords[a] - k)^2),  k = 0..15.

Sharding: S axis across the 8 NeuronCores (data parallel, one system/core).

Device algorithm (per core):
  Scatter-add via a two-level one-hot matmul. Split atom id a = q*128 + l
  (q = a >> 7 in [0,157), l = a & 127). For each tile of 128 edges build
    L[e, l] = z_e * (l == l_e)   [128 x 128] bf16  (one tensor_scalar op)
    H[e, q] = (q == q_e)         [128 x 157] bf16  (one tensor_scalar op)
  and accumulate  coords[l, q] += L^T @ H  into PSUM with the tensor engine.
  The KDE then reads coords from PSUM: 16x (Square, Exp+row-accum) on the
  scalar engine, followed by a ones-vector matmul for the partition reduce.
  Atoms 20000..20095 are phantom PSUM cells; the KDE reduction APs simply
  exclude them (q == 156 is only reduced over l < 32).
"""

import numpy as np

import concourse.tile as tile
from concourse import bacc, mybir
from concourse.bass_utils import run_bass_kernel_spmd

P = 128
NQ = 158            # ceil(20000 / 128), padded to even for DVE 4x mode
NATOMS = 20000
K = 16
E = 1_000_000
NCOL_FULL = 7813    # ceil(E / 128); E_pad = 7813*128 = 1000064
TBLK = 512          # columns (of 128 edges) per DMA block

R1 = 4.4
INV2 = float(1.0 / (1.1 * 1.1))   # 1/(R0-R1)^2
PAD_ATOM = 20064    # phantom cell (l=96, q=156)

F32 = mybir.dt.float32
BF16 = mybir.dt.bfloat16
I32 = mybir.dt.int32
OP = mybir.AluOpType
AF = mybir.ActivationFunctionType


def build_nc(ncol=NCOL_FULL):
    """Build the per-core Bass module. Inputs are padded to ncol*128 edges."""
    e_pad = ncol * P
    nc = bacc.Bacc("TRN2", target_bir_lowering=False, debug=False)
    nv = nc.dram_tensor("nv", [e_pad * 3], F32, kind="ExternalInput")
    fa = nc.dram_tensor("fa", [e_pad], I32, kind="ExternalInput")
    out = nc.dram_tensor("out", [1, K], F32, kind="ExternalOutput")

    # block layout: blocks of TBLK columns (+ remainder block)
    blocks = []
    c = 0
    while c < ncol:
        tb = min(TBLK, ncol - c)
        blocks.append((c, tb))
        c += tb

    with tile.TileContext(nc) as tc:
        with (
            tc.tile_pool(name="const", bufs=1) as cpool,
            tc.tile_pool(name="io", bufs=2) as iopool,
            tc.tile_pool(name="work", bufs=2) as wpool,
            tc.tile_pool(name="mask", bufs=4) as mpool,
            tc.tile_pool(name="psum", bufs=1, space="PSUM") as ppool,
        ):
            iota_l = cpool.tile([P, P], BF16)
            nc.gpsimd.iota(iota_l[:], pattern=[[1, P]], base=0,
                           channel_multiplier=0,
                           allow_small_or_imprecise_dtypes=True)
            iota_q = cpool.tile([P, NQ], BF16)
            nc.gpsimd.iota(iota_q[:], pattern=[[1, NQ]], base=0,
                           channel_multiplier=0,
                           allow_small_or_imprecise_dtypes=True)
            ones = cpool.tile([P, 1], F32)
            nc.vector.memset(ones[:], 1.0)
            bias_m1 = cpool.tile([P, 1], F32)
            nc.vector.memset(bias_m1[:], -1.0)
            bias_m4 = cpool.tile([P, 1], F32)
            nc.vector.memset(bias_m4[:], -4.0)
            bias_k = cpool.tile([P, K], F32)
            for k in range(K):
                nc.vector.memset(bias_k[:, k:k + 1], float(-k))

            coords = ppool.tile([P, NQ], F32, space="PSUM")

            col = 0
            for (c0, tb) in blocks:
                ofs_e = c0 * P
                nvb = iopool.tile([P, TBLK * 3], F32, tag="nvb")
                fab = iopool.tile([P, TBLK], I32, tag="fab")
                nc.sync.dma_start(
                    nvb[:, : tb * 3],
                    nv[ofs_e * 3: (ofs_e + P * tb) * 3].rearrange(
                        "(p m) -> p m", p=P),
                )
                nc.sync.dma_start(
                    fab[:, :tb],
                    fa[ofs_e: ofs_e + P * tb].rearrange("(p m) -> p m", p=P),
                )
                v3 = nvb[:, : tb * 3].rearrange("p (m c) -> p m c", c=3)
                x, y, w = v3[:, :, 0], v3[:, :, 1], v3[:, :, 2]

                d2 = wpool.tile([P, TBLK], F32, tag="d2")
                t1 = wpool.tile([P, TBLK], F32, tag="t1")
                nc.vector.tensor_tensor(out=d2[:, :tb], in0=x, in1=x, op=OP.mult)
                nc.vector.tensor_tensor(out=t1[:, :tb], in0=y, in1=y, op=OP.mult)
                nc.vector.tensor_tensor(out=d2[:, :tb], in0=d2[:, :tb],
                                        in1=t1[:, :tb], op=OP.add)
                nc.vector.tensor_tensor(out=t1[:, :tb], in0=w, in1=w, op=OP.mult)
                nc.vector.tensor_tensor(out=d2[:, :tb], in0=d2[:, :tb],
                                        in1=t1[:, :tb], op=OP.add)
                # s = sqrt(d2) / 1.1
                sv = wpool.tile([P, TBLK], F32, tag="sv")
                nc.scalar.activation(sv[:, :tb], d2[:, :tb], AF.Sqrt, scale=INV2)
                # yc = clamp(s - 4, 0, 1): relu on ACT, min on DVE
                y0 = wpool.tile([P, TBLK], F32, tag="y0")
                nc.scalar.activation(y0[:, :tb], sv[:, :tb], AF.Relu,
                                     bias=bias_m4[:])
                yc = wpool.tile([P, TBLK], F32, tag="yc")
                nc.vector.tensor_scalar(yc[:, :tb], y0[:, :tb], 1.0, None,
                                        op0=OP.min)
                # z = (yc-1)^2 * (2*yc + 1)
                vv = wpool.tile([P, TBLK], F32, tag="vv")
                nc.scalar.activation(vv[:, :tb], yc[:, :tb], AF.Square,
                                     bias=bias_m1[:])
                w2 = wpool.tile([P, TBLK], F32, tag="w2")
                nc.vector.tensor_scalar(w2[:, :tb], yc[:, :tb], 2.0, 1.0,
                                        op0=OP.mult, op1=OP.add)
                zf = wpool.tile([P, TBLK], F32, tag="zf")
                nc.vector.tensor_tensor(out=zf[:, :tb], in0=vv[:, :tb],
                                        in1=w2[:, :tb], op=OP.mult)
                # q = a >> 7, l = a & 127 (as f32 scalars for is_equal)
                qi = wpool.tile([P, TBLK], I32, tag="qi")
                ri = wpool.tile([P, TBLK], I32, tag="ri")
                nc.vector.tensor_scalar(qi[:, :tb], fab[:, :tb], 7, None,
                                        op0=OP.logical_shift_right)
                nc.vector.tensor_scalar(ri[:, :tb], fab[:, :tb], 127, None,
                                        op0=OP.bitwise_and)
                qf = wpool.tile([P, TBLK], F32, tag="qf")
                rf = wpool.tile([P, TBLK], F32, tag="rf")
                nc.vector.tensor_copy(qf[:, :tb], qi[:, :tb])
                nc.vector.tensor_copy(rf[:, :tb], ri[:, :tb])
                # negated q, used as the ACT Square bias for ACT-built H masks
                nqf = wpool.tile([P, TBLK], F32, tag="nqf")
                nc.vector.tensor_scalar(nqf[:, :tb], qf[:, :tb], -1.0, None,
                                        op0=OP.mult)

                # process columns in groups of GRP; masks live in per-group
                # mega-tiles so pool slot-waits amortize across the group.
                # Engine split per 27-col group (load-balance equilibrium
                # under the instruction cost model):
                #   L masks: 13 cols DVE, 14 cols GPSIMD
                #   H masks: 22 cols DVE, 5 cols ACT (2-op one-hot)
                GRP = 27
                t = 0
                while t < tb:
                    g = min(GRP, tb - t)
                    lmega = mpool.tile([P, GRP * P], BF16, tag="lmega")
                    hmega = mpool.tile([P, GRP * NQ], BF16, tag="hmega")
                    hsq = mpool.tile([P, 5 * NQ], BF16, tag="hsq")
                    for i in range(g):
                        lslice = lmega[:, i * P:(i + 1) * P]
                        if i < 13:
                            nc.vector.tensor_scalar(
                                lslice, iota_l[:],
                                rf[:, t + i:t + i + 1], zf[:, t + i:t + i + 1],
                                op0=OP.is_equal, op1=OP.mult)
                        else:
                            nc.gpsimd.tensor_scalar(
                                lslice, iota_l[:],
                                rf[:, t + i:t + i + 1], zf[:, t + i:t + i + 1],
                                op0=OP.is_equal, op1=OP.mult)
                        hslice = hmega[:, i * NQ:(i + 1) * NQ]
                        if i >= 5:
                            nc.vector.tensor_scalar(
                                hslice, iota_q[:], qf[:, t + i:t + i + 1],
                                None, op0=OP.is_equal)
                        else:
                            # one-hot via (q-qe)^2 -> relu(1-sq); exact for ints
                            hsq_s = hsq[:, i * NQ:(i + 1) * NQ]
                            nc.scalar.activation(hsq_s, iota_q[:], AF.Square,
                                                 bias=nqf[:, t + i:t + i + 1])
                            nc.scalar.activation(hslice, hsq_s, AF.Relu,
                                                 bias=ones[:], scale=-1.0)
                    for i in range(g):
                        nc.tensor.matmul(
                            out=coords[:],
                            lhsT=lmega[:, i * P:(i + 1) * P],
                            rhs=hmega[:, i * NQ:(i + 1) * NQ],
                            start=(col + i == 0), stop=(col + i == ncol - 1))
                    col += g
                    t += g

            # ---- KDE ----
            acc1 = cpool.tile([P, K], F32)
            acc2 = cpool.tile([32, K], F32)
            sq = wpool.tile([P, NQ], F32, tag="sq")
            ek = wpool.tile([P, NQ], F32, tag="ek")
            for k in range(K):
                nc.scalar.activation(sq[:], coords[:], AF.Square,
                                     bias=bias_k[:, k:k + 1])
                nc.scalar.activation(ek[:, :156], sq[:, :156], AF.Exp,
                                     scale=-2.0, accum_out=acc1[:, k:k + 1])
                nc.scalar.activation(ek[0:32, 156:157], sq[0:32, 156:157],
                                     AF.Exp, scale=-2.0,
                                     accum_out=acc2[:, k:k + 1])
            hist_ps = ppool.tile([1, K], F32, space="PSUM")
            nc.tensor.matmul(out=hist_ps[:], lhsT=ones[:], rhs=acc1[:],
                             start=True, stop=False)
            nc.tensor.matmul(out=hist_ps[:], lhsT=ones[0:32, :], rhs=acc2[:],
                             start=False, stop=True)
            res = cpool.tile([1, K], F32)
            nc.vector.tensor_copy(res[:], hist_ps[:])
            nc.sync.dma_start(out[:], res[:])
    nc.compile()
    return nc


def _shard_inputs(neighbor_vectors, first_atom, ncol=NCOL_FULL):
    """Slice per system, pad to ncol*128 edges. Padding: z=0 vector (d=10)
    and a phantom atom id, so padded edges contribute exactly nothing."""
    e_pad = ncol * P
    s = neighbor_vectors.shape[0]
    in_maps = []
    for i in range(s):
        nvs = np.asarray(neighbor_vectors[i], dtype=np.float32).reshape(-1, 3)
        fas = np.asarray(first_atom[i], dtype=np.int32).reshape(-1)
        n = min(e_pad, nvs.shape[0])
        nv_pad = np.empty((e_pad, 3), dtype=np.float32)
        nv_pad[:n] = nvs[:n]
        nv_pad[n:] = np.array([10.0, 0.0, 0.0], dtype=np.float32)
        fa_pad = np.full((e_pad,), PAD_ATOM, dtype=np.int32)
        fa_pad[:n] = fas[:n]
        in_maps.append({"nv": nv_pad.reshape(-1), "fa": fa_pad})
    return in_maps


def run(neighbor_vectors, first_atom, ncol=NCOL_FULL, trace=False):
    nc = build_nc(ncol)
    in_maps = _shard_inputs(neighbor_vectors, first_atom, ncol)
    br = run_bass_kernel_spmd(nc, in_maps, core_ids=list(range(len(in_maps))),
                              trace=trace)
    out = np.stack([br.results[i]["out"][0] for i in range(len(in_maps))])
    return out.astype(np.float32), br


def kernel(neighbor_vectors, first_atom):
    out, _ = run(neighbor_vectors, first_atom)
    return out
